# revision 1
# baseline (speedup 1.0000x reference)
"""Trainium2 Bass kernel for the retrieval-KNN correlation problem.

Problem (per batch element b):
    idx[k,p]   = x[b,k,p] + 64*y[b,k,p]              (pixel coords into ref map)
    S[k,p]     = sum_c ref[b,c,idx[k,p]] * inp[b,c,p]
    best[p]    = argmax_k S[k,p]        (first occurrence on ties)
    out_x[p]   = x[b,best[p],p],  out_y[p] = y[b,best[p],p]

Sharding: 8 cores = (batch b = core//2, pixel half = core%2); no cross-core
communication.

The Q7 ap_gather costs ~26ns/index + ~1.25ns/element (HW-measured), so the
index count dominates. ref is packed d=8 (8 channels per gathered element
row; a channel set spans 32 partitions, replicated 4x across the partition
dim), so ONE gather instruction serves FOUR candidates at once - half the
index wall of the d=4 baseline (295us vs 508us per core). The argmax margins
in this problem (min effective top-2 gap 1.7e-5) force true fp32 throughout:
fp32r matmul is TF32-grade and 16-bit features flip argmaxes (measured).

Per-core dataflow, 4 pixel-chunks of 512 pixels:
  - 4 ap_gathers per chunk (one per candidate quad, num_idxs=512), indices
    prebuilt on host (no on-device index pipeline).
  - one contiguous in-place DVE multiply per gather against the d=8-packed
    inp chunk (streamed from HBM, double buffered), then a DVE segment
    reduce over d=8 and ONE 512-col fp32 matmul per quad with one-hot
    weights accumulates the partition-dim channel reduction into a (16,512)
    PSUM tile. (Strided-rhs fp32 matmuls straight off the interleaved
    product run at 2.1x cycles/col and stall the gather pipeline - measured.)
  - per-chunk epilogue: PE-transpose S to pixel-major, DVE argmax with the
    reverse-weight first-occurrence trick, select x/y; overlaps the next
    chunk's gathers.
  - reduce tiles live in their own pool (sharing the gather pool's ring
    serialized gather launches behind reduces); xt/yt/revc loads are issued
    after chunk 0's gathers to stay out of the first gather's DMA wait set.

HW-verified: exact match vs the jax reference (rel err 0.0), 415.8us traced
(best; ~417-440us across runs) vs the 630us d=4 baseline.
"""

import numpy as np
from contextlib import ExitStack

import concourse.bacc as bacc
import concourse.bass as bass
import concourse.mybir as mybir
import concourse.tile as tile
from concourse import bass_utils

B, K, CN, H, W = 4, 16, 256, 64, 64
HW = H * W
HALF = HW // 2        # 2048 pixels per core
NCORES = 8
NS = HALF // 128      # 16 output slots
CHUNK = 512
NCH = HALF // CHUNK   # 4 chunks
NQ = K // 4           # 4 candidate quads
D = 8

f32 = mybir.dt.float32
i16 = mybir.dt.int16


def build_program():
    nc = bacc.Bacc("TRN2", target_bir_lowering=False, debug=False)

    ref8_d = nc.dram_tensor("ref8", (128, D * HW), f32, kind="ExternalInput")
    inp8_d = nc.dram_tensor("inp8", (128, D * HALF), f32, kind="ExternalInput")
    wk_d = nc.dram_tensor("wk", (128, NCH * NQ * (CHUNK // 16)), i16,
                          kind="ExternalInput")
    wq_d = nc.dram_tensor("wq", (128, NQ * K), f32, kind="ExternalInput")
    ident_d = nc.dram_tensor("ident", (16, 16), f32, kind="ExternalInput")
    xt_d = nc.dram_tensor("xt", (128, NS * K), f32, kind="ExternalInput")
    yt_d = nc.dram_tensor("yt", (128, NS * K), f32, kind="ExternalInput")
    revc_d = nc.dram_tensor("revc", (128, NS * K), f32, kind="ExternalInput")
    ox_d = nc.dram_tensor("ox", (128, NS), f32, kind="ExternalOutput")
    oy_d = nc.dram_tensor("oy", (128, NS), f32, kind="ExternalOutput")

    IW = CHUNK // 16  # 32 idx words per partition per (chunk, quad)

    with ExitStack() as ctx:
        tc = ctx.enter_context(tile.TileContext(nc))
        pers = ctx.enter_context(tc.tile_pool(name="pers", bufs=1))
        gpool = ctx.enter_context(tc.tile_pool(name="g", bufs=2))
        # red in its own pool: sharing gpool's ring made the next gather's
        # launch wait on the previous quad's reduce (trace-verified +2.1us
        # bubble on alternating gathers).
        rpool = ctx.enter_context(tc.tile_pool(name="r", bufs=2))
        ipool = ctx.enter_context(tc.tile_pool(name="i", bufs=2))
        spool = ctx.enter_context(tc.tile_pool(name="s", bufs=2))
        ps_s = ctx.enter_context(tc.tile_pool(name="ps_s", bufs=2, space="PSUM"))
        ps_tp = ctx.enter_context(tc.tile_pool(name="ps_tp", bufs=2, space="PSUM"))

        ref8 = pers.tile([128, D * HW], f32, tag="ref8")
        wk = pers.tile([128, NCH * NQ * IW], i16, tag="wk")
        wq = pers.tile([128, NQ * K], f32, tag="wq")
        ident = pers.tile([16, 16], f32, tag="ident")
        xt = pers.tile([128, NS * K], f32, tag="xt")
        yt = pers.tile([128, NS * K], f32, tag="yt")
        revc = pers.tile([128, NS * K], f32, tag="revc")
        oxv = pers.tile([128, NS], f32, tag="oxv")
        oyv = pers.tile([128, NS], f32, tag="oyv")

        nc.sync.dma_start(wk[:], wk_d.ap())
        nc.sync.dma_start(wq[:], wq_d.ap())
        nc.sync.dma_start(ident[:], ident_d.ap())
        # ref8 is 16MB - split across 4 DMAs so the transfers parallelize
        # over DMA engines instead of serializing on one queue. (Issuing from
        # other engines was tried and is slower: a gpsimd-issued DMA
        # serializes against the Pool gather stream.)
        QTR = D * HW // 4
        for part in range(4):
            nc.sync.dma_start(ref8[:, QTR * part:QTR * (part + 1)],
                              ref8_d.ap()[:, QTR * part:QTR * (part + 1)])

        for c in range(NCH):
            i8 = ipool.tile([128, D * CHUNK], f32, tag="i8", name=f"i8_{c}")
            nc.sync.dma_start(
                i8[:], inp8_d.ap()[:, D * CHUNK * c:D * CHUNK * (c + 1)])

            s_ps = ps_s.tile([K, CHUNK], f32, tag="s", name=f"s{c}")
            for q in range(NQ):
                gb = gpool.tile([128, D * CHUNK], f32, tag="gb",
                                name=f"gb{c}_{q}")
                nc.gpsimd.ap_gather(
                    gb[:].rearrange("p (i d) -> p i d", d=D),
                    ref8[:].rearrange("p (e d) -> p e d", d=D),
                    wk[:, (c * NQ + q) * IW:(c * NQ + q + 1) * IW],
                    channels=128, num_elems=HW, d=D, num_idxs=CHUNK)
                nc.vector.tensor_mul(gb[:], gb[:], i8[:])
                red = rpool.tile([128, CHUNK], f32, tag="red",
                                 name=f"red{c}_{q}")
                nc.vector.tensor_reduce(
                    red[:], gb[:].rearrange("p (i d) -> p i d", d=D),
                    axis=mybir.AxisListType.X, op=mybir.AluOpType.add)
                nc.tensor.matmul(
                    s_ps[:, :],
                    lhsT=wq[:, K * q:K * (q + 1)],
                    rhs=red[:],
                    start=(q == 0),
                    stop=(q == NQ - 1),
                )

            if c == 0:
                # xt/yt/revc are first needed by this epilogue; issuing them
                # here keeps them out of the first gather's DMA wait set.
                nc.sync.dma_start(xt[:], xt_d.ap())
                nc.sync.dma_start(yt[:], yt_d.ap())
                nc.sync.dma_start(revc[:], revc_d.ap())

            # ---- chunk epilogue: transpose S to pixel-major, argmax -------
            ssb = spool.tile([K, CHUNK], f32, tag="ssb", name=f"ssb{c}")
            nc.scalar.copy(ssb[:], s_ps[:])
            st = spool.tile([128, 4 * K], f32, tag="st", name=f"st{c}")
            for t in range(4):
                stp = ps_tp.tile([128, K], f32, tag="tp", name=f"tp{c}_{t}")
                nc.tensor.transpose(stp[:], ssb[:, 128 * t:128 * (t + 1)],
                                    ident[:])
                nc.scalar.copy(st[:, K * t:K * (t + 1)], stp[:])

            def grp(ap):
                return ap.rearrange("p (t k) -> p t k", k=K)

            gmax = spool.tile([128, 4], f32, tag="gmax", name=f"gm{c}")
            ohall = spool.tile([128, 4 * K], f32, tag="ohall", name=f"oh{c}")
            t1 = spool.tile([128, 4 * K], f32, tag="t1", name=f"t1{c}")
            r1 = spool.tile([128, 4], f32, tag="r1", name=f"r1{c}")
            oh1 = spool.tile([128, 4 * K], f32, tag="oh1", name=f"oh1{c}")
            sel = spool.tile([128, 4 * K], f32, tag="sel", name=f"sel{c}")

            xs = xt[:, 4 * K * c:4 * K * (c + 1)]
            ys = yt[:, 4 * K * c:4 * K * (c + 1)]
            rs = revc[:, 4 * K * c:4 * K * (c + 1)]

            nc.vector.tensor_reduce(gmax[:], grp(st[:]),
                                    axis=mybir.AxisListType.X,
                                    op=mybir.AluOpType.max)
            gb_ = gmax[:].unsqueeze(2).broadcast_to((128, 4, K))
            nc.vector.tensor_tensor(grp(ohall[:]), grp(st[:]), gb_,
                                    op=mybir.AluOpType.is_equal)
            nc.vector.tensor_mul(t1[:], ohall[:], rs)
            nc.vector.tensor_reduce(r1[:], grp(t1[:]),
                                    axis=mybir.AxisListType.X,
                                    op=mybir.AluOpType.max)
            rb = r1[:].unsqueeze(2).broadcast_to((128, 4, K))
            nc.vector.tensor_tensor(grp(oh1[:]), grp(t1[:]), rb,
                                    op=mybir.AluOpType.is_equal)
            nc.vector.tensor_mul(sel[:], oh1[:], xs)
            nc.vector.tensor_reduce(oxv[:, 4 * c:4 * (c + 1)], grp(sel[:]),
                                    axis=mybir.AxisListType.X,
                                    op=mybir.AluOpType.add)
            nc.vector.tensor_mul(sel[:], oh1[:], ys)
            nc.vector.tensor_reduce(oyv[:, 4 * c:4 * (c + 1)], grp(sel[:]),
                                    axis=mybir.AxisListType.X,
                                    op=mybir.AluOpType.add)

        nc.sync.dma_start(ox_d.ap(), oxv[:])
        nc.sync.dma_start(oy_d.ap(), oyv[:])

    nc.compile()
    return nc


def pack_oct(a, n):
    """(256, n) -> (128, 8n): partition p holds channels 8*(p%32)+j
    interleaved per pixel, replicated 4x across partition quadrants."""
    r = a.reshape(32, 8, n).transpose(0, 2, 1).reshape(32, 8 * n)
    return np.ascontiguousarray(np.tile(r, (4, 1)))


def wrap_chunk(flat):
    """(CHUNK,) -> (16, CHUNK//16) wrapped: w[j%16, j//16] = flat[j]."""
    w = np.empty((16, CHUNK // 16), dtype=np.int16)
    w[np.arange(CHUNK) % 16, np.arange(CHUNK) // 16] = flat
    return w


def pixel_major(a):
    rows = a.shape[0]
    return np.ascontiguousarray(a.T.reshape(NS, 128, rows).transpose(1, 0, 2))


def make_in_maps(input_features, ref_features, aggregated_x, aggregated_y):
    revc = np.tile(
        np.tile(np.arange(K, 0, -1, dtype=np.float32), NS).reshape(1, NS * K),
        (128, 1))
    # wq[p, 16q + m] = 1 iff m == 4q + p//32
    wq = np.zeros((128, NQ * K), dtype=np.float32)
    for q in range(NQ):
        for blk in range(4):
            wq[32 * blk:32 * (blk + 1), K * q + 4 * q + blk] = 1.0
    ident = np.eye(16, dtype=np.float32)
    in_maps = []
    ref8_cache = {}
    for core in range(NCORES):
        b, h = core // 2, core % 2
        sl = slice(h * HALF, (h + 1) * HALF)
        if b not in ref8_cache:
            ref8_cache[b] = pack_oct(ref_features[b].reshape(CN, HW), HW)
        x = aggregated_x[b].reshape(K, HW)[:, sl]
        y = aggregated_y[b].reshape(K, HW)[:, sl]
        idx = (x + y * W).astype(np.int64)  # (K, HALF)
        wkblocks = []
        for c in range(NCH):
            for q in range(NQ):
                blk = np.empty((128, CHUNK // 16), dtype=np.int16)
                for g in range(8):
                    cand = 4 * q + g // 2
                    blk[16 * g:16 * (g + 1)] = wrap_chunk(
                        idx[cand, CHUNK * c:CHUNK * (c + 1)])
                wkblocks.append(blk)
        in_maps.append({
            "ref8": ref8_cache[b],
            "inp8": pack_oct(input_features[b].reshape(CN, HW)[:, sl], HALF),
            "wk": np.ascontiguousarray(np.concatenate(wkblocks, axis=1)),
            "wq": wq,
            "ident": ident,
            "xt": pixel_major(x).reshape(128, NS * K),
            "yt": pixel_major(y).reshape(128, NS * K),
            "revc": revc,
        })
    return in_maps


def assemble_outputs(results):
    offset_x = np.empty((B, 1, H, W), dtype=np.float32)
    offset_y = np.empty((B, 1, H, W), dtype=np.float32)
    for core in range(NCORES):
        b, h = core // 2, core % 2
        sl = slice(h * HALF, (h + 1) * HALF)
        offset_x[b, 0].reshape(HW)[sl] = results[core]["ox"].T.reshape(HALF)
        offset_y[b, 0].reshape(HW)[sl] = results[core]["oy"].T.reshape(HALF)
    return offset_x, offset_y


_PROGRAM = None


def kernel(input_features, ref_features, aggregated_x, aggregated_y):
    global _PROGRAM
    if _PROGRAM is None:
        _PROGRAM = build_program()
    nc = _PROGRAM
    in_maps = make_in_maps(input_features, ref_features, aggregated_x, aggregated_y)
    res = bass_utils.run_bass_kernel_spmd(nc, in_maps, core_ids=list(range(NCORES)))
    return assemble_outputs(res.results)

